# revision 48
# baseline (speedup 1.0000x reference)
# Local (sliding-window, strictly-causal) multi-head attention for Trainium2.
#
# Problem: nn_LocalAttention  (B=2, S=4096, MD=AD=1024, NH=8, HD=128, window=256)
#   q = query @ Wq.T ; per-head scores q.k/sqrt(HD) masked to col in [row-256, row-1];
#   softmax; out = w @ v ; rows with no valid keys zeroed; out @ Wo.T.
#
# Sharding (8 cores): batch (2) x sequence chunks (4 x 1024 rows).  Each core runs
# the whole pipeline for its 1024 query rows using a 256-row K/V halo, so the 8
# output shards are disjoint and the gather is pure concatenation.  Weights are
# replicated.  All tensors are pre-laid-out host-side in their exact SBUF
# [partition, ...] shape (bf16), so every DMA is a single contiguous transfer.
#
# All matmuls run in bf16 (fp32 matmuls execute as two HW passes on TRN2 and
# forgo fast-weight-load; bf16 halves PE time and DMA bytes).  PSUM accumulation
# stays fp32.  Masking is multiplicative on the exp output (DVE) instead of
# additive -1e5 bias matmuls, which removes 144 PE matmuls.
#
# Structure: ONE pass per head — Wq projection for head h (both C-halves), then
# that head's 8 attention query tiles — followed by a single Wo phase.  This
# keeps the PE stream dense from ~10us on (no phase-1/phase-2 boundary) and
# makes every DMA deadline lenient.  DMAs are explicitly ordered with dep edges
# so early-needed transfers are not bandwidth-starved by later ones.
#
# Engine split per (head, qtile): PE 6 matmuls + 1 transpose; ACT exp +
# normalize-copy (Copy activation, per-partition scale = 1/denominator); DVE
# mask multiply + reciprocal + outT copy; GPSIMD idle (its tensor ops measured
# ~3x slower than DVE and stalled the PV critical path).

import math

import ml_dtypes
import numpy as np

import concourse.bass as bass
import concourse.tile as tile
from concourse import bacc, mybir
from concourse.bass_utils import run_bass_kernel_spmd
from concourse.masks import make_identity
from concourse.tile_rust import add_dep_helper

F32 = mybir.dt.float32
BF16 = mybir.dt.bfloat16
NPBF16 = ml_dtypes.bfloat16

NH = 8       # heads
HD = 128     # head dim
B = 2        # batch
S = 4096     # sequence
MD = 1024    # model dim
AD = 1024    # attn dim
WIN = 256    # window
C = 1024     # query rows per core (chunk)
NQT = C // 128          # 8 query tiles per chunk
HALO = WIN + C          # 1280 key/value rows per core
NKB = HALO // 128       # 10 key blocks
NCORES = 8


# ----------------------------------------------------------------------------
# device program
# ----------------------------------------------------------------------------

def _emit(ctx, tc: tile.TileContext, qc, wq, wo, kT, vp, msk, out):
    nc = tc.nc

    const_pool = ctx.enter_context(tc.tile_pool(name="const", bufs=1))
    ident = const_pool.tile([128, 128], F32)
    make_identity(nc, ident)

    kT_pool = ctx.enter_context(tc.tile_pool(name="kT", bufs=1))
    msk_pool = ctx.enter_context(tc.tile_pool(name="msk", bufs=1))
    qT_pool = ctx.enter_context(tc.tile_pool(name="qT", bufs=1))
    vp_pool = ctx.enter_context(tc.tile_pool(name="vp", bufs=1))
    wo_pool = ctx.enter_context(tc.tile_pool(name="wo", bufs=1))
    outT_pool = ctx.enter_context(tc.tile_pool(name="outT", bufs=1))
    qc_pool = ctx.enter_context(tc.tile_pool(name="qc", bufs=1))
    wq_pool = ctx.enter_context(tc.tile_pool(name="wq", bufs=1))

    qc_sb = qc_pool.tile([128, 2, 8, 512], BF16)
    wq_sb = wq_pool.tile([128, NH, 8, 128], BF16)
    kT_sb = kT_pool.tile([128, NH, HALO], BF16)
    msk_sb = msk_pool.tile([128, NKB, 3, 128], BF16)
    vp_sb = vp_pool.tile([128, NH, NKB, HD + 1], BF16)
    wo_sb = wo_pool.tile([128, NH, MD], BF16)
    qT_sb = qT_pool.tile([128, NH, C], BF16)
    outT_sb = outT_pool.tile([128, NH, NQT, 128], BF16)

    # DMAs, dependency-chained so arrival order tracks the compute deadlines:
    #   {qc half0, wq h0} -> qc half1 -> msk -> kv_h0 -> wq h1-7 -> kv_h1 -> ...
    d_qc0 = nc.sync.dma_start(out=qc_sb[:, 0], in_=qc[:, 0])
    d_wq0 = nc.sync.dma_start(out=wq_sb[:, 0:1], in_=wq[:, 0:1])
    d_qc1 = nc.sync.dma_start(out=qc_sb[:, 1], in_=qc[:, 1])
    d_msk = nc.sync.dma_start(out=msk_sb, in_=msk)
    d_kv = []
    for h in range(NH):
        d_k = nc.sync.dma_start(out=kT_sb[:, h], in_=kT[:, h])
        d_v = nc.sync.dma_start(out=vp_sb[:, h], in_=vp[:, h])
        d_kv.append((d_k, d_v))
    d_wq1 = nc.sync.dma_start(out=wq_sb[:, 1:2], in_=wq[:, 1:2])
    d_wqr = nc.sync.dma_start(out=wq_sb[:, 2:NH], in_=wq[:, 2:NH])
    d_wo = nc.sync.dma_start(out=wo_sb, in_=wo)

    chain = [[d_qc1, d_wq1], list(d_kv[0]) + [d_msk], [d_wqr] + list(d_kv[1]),
             list(d_kv[2]) + list(d_kv[3]), list(d_kv[4]) + list(d_kv[5]),
             list(d_kv[6]) + list(d_kv[7]), [d_wo]]
    prev = [d_qc0, d_wq0]
    for grp in chain:
        for dn in grp:
            for dp in prev:
                add_dep_helper(dn.ins, dp.ins, True, "dma deadline order")
        prev = grp

    # ---------------- per-head: Wq projection then attention -----------------
    with tc.tile_pool(name="qp_psum", bufs=2, space="PSUM") as qp_psum, \
         tc.tile_pool(name="sc_psum", bufs=2, space="PSUM") as sc_psum, \
         tc.tile_pool(name="ov_psum", bufs=2, space="PSUM") as ov_psum, \
         tc.tile_pool(name="tr_psum", bufs=2, space="PSUM") as tr_psum, \
         tc.tile_pool(name="e", bufs=4) as e_pool, \
         tc.tile_pool(name="em", bufs=2) as em_pool, \
         tc.tile_pool(name="oh", bufs=3) as oh_pool, \
         tc.tile_pool(name="r", bufs=3) as r_pool:

        for h in range(NH):
            for half in range(2):
                ps = qp_psum.tile([128, 512], F32)
                for mt in range(8):
                    nc.tensor.matmul(
                        ps,
                        lhsT=wq_sb[:, h, mt, :],
                        rhs=qc_sb[:, half, mt, :],
                        start=(mt == 0),
                        stop=(mt == 7),
                    )
                if half == 0:
                    nc.scalar.copy(
                        qT_sb[:, h, half * 512:(half + 1) * 512], ps)
                else:
                    nc.vector.tensor_copy(
                        qT_sb[:, h, half * 512:(half + 1) * 512], ps)

            # scores block-major: one matmul per key block j covers every
            # query tile that looks at it (qt in [j-2, j]), amortizing the
            # kT weight load 3x and cutting the matmul count 24 -> 10 per head.
            em_all = em_pool.tile([128, NKB, 3, 128], BF16)
            for j in range(NKB):
                lo = max(0, j - 2)
                hi = min(NQT, j + 1)
                width = (hi - lo) * 128
                s_ps = sc_psum.tile([128, 3 * 128], F32, tag="s")
                nc.tensor.matmul(
                    s_ps[:, 0:width],
                    lhsT=kT_sb[:, h, j * 128:(j + 1) * 128],
                    rhs=qT_sb[:, h, lo * 128:hi * 128],
                    start=True,
                    stop=True,
                )
                e_sb = e_pool.tile([128, 3 * 128], BF16, tag="e")
                nc.scalar.activation(
                    e_sb[:, 0:width],
                    s_ps[:, 0:width],
                    mybir.ActivationFunctionType.Exp,
                )
                nc.vector.tensor_mul(
                    em_all[:, j].rearrange("p a b -> p (a b)")[:, 0:width],
                    e_sb[:, 0:width],
                    msk_sb[:, j].rearrange("p a b -> p (a b)")[:, 0:width],
                )

            def flush(pqt, poh):
                # transpose of tile pqt, emitted one iteration late so the PE
                # never waits on the recip/normalize chain of the same tile
                t_ps = tr_psum.tile([128, 128], F32, name="t_ps", tag="t")
                nc.tensor.transpose(t_ps, poh, ident)
                nc.vector.tensor_copy(outT_sb[:, h, pqt, :], t_ps)

            pending = None
            for qt in range(NQT):
                # out[t, d] plus the softmax denominator in column 128
                o_ps = ov_psum.tile([128, HD + 1], F32)
                for sub in range(3):
                    j = qt + sub
                    idx = qt - max(0, j - 2)
                    nc.tensor.matmul(
                        o_ps,
                        lhsT=em_all[:, j, idx, :],
                        rhs=vp_sb[:, h, j, :],
                        start=(sub == 0),
                        stop=(sub == 2),
                    )
                r_sb = r_pool.tile([128, 1], F32)
                nc.vector.reciprocal(r_sb, o_ps[:, HD:HD + 1])
                oh_sb = oh_pool.tile([128, 128], F32)
                if qt % 2 == 0:
                    nc.scalar.activation(
                        oh_sb,
                        o_ps[:, 0:HD],
                        mybir.ActivationFunctionType.Copy,
                        scale=r_sb[:, 0:1],
                    )
                else:
                    nc.vector.tensor_scalar_mul(oh_sb, o_ps[:, 0:HD], r_sb)
                if pending is not None:
                    flush(*pending)
                pending = (qt, oh_sb)
            flush(*pending)

    # ---------------- output projection --------------------------------------
    with tc.tile_pool(name="stage", bufs=2) as stage_pool, \
         tc.tile_pool(name="fi_psum", bufs=2, space="PSUM") as fi_psum:
        for qt in range(NQT):
            f_ps = fi_psum.tile([128, MD], F32)
            for h in range(NH):
                lhsT = outT_sb[:, h, qt, :]
                for nn in range(2):
                    nc.tensor.matmul(
                        f_ps[:, nn * 512:(nn + 1) * 512],
                        lhsT=lhsT,
                        rhs=wo_sb[:, h, nn * 512:(nn + 1) * 512],
                        start=(h == 0),
                        stop=(h == NH - 1),
                    )
            st = stage_pool.tile([128, MD], BF16)
            nc.vector.tensor_copy(st, f_ps)
            nc.sync.dma_start(out=out[qt * 128:(qt + 1) * 128, :], in_=st)


_CACHED_NC = {}


def _build_program():
    if "nc" in _CACHED_NC:
        return _CACHED_NC["nc"]
    nc = bacc.Bacc("TRN2", target_bir_lowering=False, debug=False)
    qc = nc.dram_tensor("qc", [128, 2, 8, 512], BF16, kind="ExternalInput").ap()
    wq = nc.dram_tensor("wq", [128, NH, 8, 128], BF16, kind="ExternalInput").ap()
    wo = nc.dram_tensor("wo", [128, NH, MD], BF16, kind="ExternalInput").ap()
    kT = nc.dram_tensor("kT", [128, NH, HALO], BF16, kind="ExternalInput").ap()
    vp = nc.dram_tensor("vp", [128, NH, NKB, HD + 1], BF16, kind="ExternalInput").ap()
    msk = nc.dram_tensor("msk", [128, NKB, 3, 128], BF16, kind="ExternalInput").ap()
    out = nc.dram_tensor("out", [C, MD], BF16, kind="ExternalOutput").ap()
    from contextlib import ExitStack

    with tile.TileContext(nc) as tc:
        with ExitStack() as ctx:
            _emit(ctx, tc, qc, wq, wo, kT, vp, msk, out)
    nc.compile()
    _CACHED_NC["nc"] = nc
    return nc


# ----------------------------------------------------------------------------
# host-side shard construction
# ----------------------------------------------------------------------------

def _build_mask(s0: int) -> np.ndarray:
    """Multiplicative 0/1 window mask, block-major: [k, kblock j, slot, t].

    Block j's score matmul covers query tiles qt in [max(0, j-2), min(8, j+1));
    slot idx maps to qt = max(0, j-2) + idx."""
    m = np.zeros((128, NKB, 3, 128), np.float32)
    tt = np.arange(128)
    kk = np.arange(128)
    for j in range(NKB):
        lo = max(0, j - 2)
        hi = min(NQT, j + 1)
        k_abs = s0 + j * 128 - WIN + kk                 # [k]
        for idx in range(hi - lo):
            qt = lo + idx
            q_abs = s0 + qt * 128 + tt                  # [t]
            valid = (
                (k_abs[:, None] < q_abs[None, :])
                & (q_abs[None, :] - k_abs[:, None] <= WIN)
                & (k_abs[:, None] >= 0)
            )
            m[:, j, idx, :][valid] = 1.0
    if s0 == 0:
        # row 0 has no valid keys; give it one unmasked zero-padding key so
        # softmax yields weight 1 on v=0 -> output row is exactly 0, matching
        # the reference's has_valid zeroing.  (qt=0, sub=0 -> block 0, slot 0.)
        m[0, 0, 0, 0] = 1.0
    return m.astype(NPBF16)


def _make_in_maps(query_seq, keys_seq, values_seq, Wq, Wo):
    q = np.asarray(query_seq, dtype=np.float32)
    k = np.asarray(keys_seq, dtype=np.float32)
    v = np.asarray(values_seq, dtype=np.float32)
    wq_f = np.asarray(Wq, dtype=np.float32)
    wo_f = np.asarray(Wo, dtype=np.float32)

    scale = np.float32(math.sqrt(float(HD)))
    # wq[p, h, mt, j] = Wq[h*128+j, mt*128+p] / scale  (h-major for early start)
    wq_h = np.ascontiguousarray(
        (wq_f.T / scale).reshape(8, 128, NH, 128).transpose(1, 2, 0, 3)
    ).astype(NPBF16)
    # wo[d, h, o] = Wo[o, h*128+d]
    wo_h = np.ascontiguousarray(
        wo_f.T.reshape(NH, 128, MD).transpose(1, 0, 2)).astype(NPBF16)

    masks = {0: _build_mask(0)}

    in_maps = []
    for core in range(NCORES):
        b, ch = divmod(core, S // C)
        s0 = ch * C

        # qc[p, half, mt, t'] = q[b, s0+half*512+t', mt*128+p]
        qc_h = np.ascontiguousarray(
            q[b, s0:s0 + C, :].reshape(2, 512, 8, 128).transpose(3, 0, 2, 1)
        ).astype(NPBF16)

        khalo = np.zeros((HALO, AD), np.float32)
        vhalo = np.zeros((HALO, AD), np.float32)
        lo = s0 - WIN
        off = max(0, -lo)
        khalo[off:] = k[b, lo + off:s0 + C, :]
        vhalo[off:] = v[b, lo + off:s0 + C, :]

        # kT[d, h, j] = khalo[j, h*128+d]
        kT_h = np.ascontiguousarray(
            khalo.reshape(HALO, NH, HD).transpose(2, 1, 0)).astype(NPBF16)

        # vp[p, h, blk, d] = vhalo[blk*128+p, h*128+d]; ones at d=HD
        vp_h = np.ones((128, NH, NKB, HD + 1), np.float32)
        vp_h[:, :, :, :HD] = vhalo.reshape(NKB, 128, NH, HD).transpose(1, 2, 0, 3)
        vp_h = np.ascontiguousarray(vp_h).astype(NPBF16)

        if s0 not in masks:
            masks[s0] = _build_mask(s0)

        in_maps.append({
            "qc": qc_h,
            "wq": wq_h,
            "wo": wo_h,
            "kT": kT_h,
            "vp": vp_h,
            "msk": masks[s0],
        })
    return in_maps


def _gather(results) -> np.ndarray:
    out = np.empty((B, S, MD), np.float32)
    for core in range(NCORES):
        b, ch = divmod(core, S // C)
        out[b, ch * C:(ch + 1) * C, :] = results[core]["out"].astype(np.float32)
    return out


def _run(in_maps, **kwargs):
    nc = _build_program()
    return run_bass_kernel_spmd(nc, in_maps, list(range(NCORES)), **kwargs)


def kernel(query_seq, keys_seq, values_seq, Wq, Wo, window=WIN, **_unused):
    assert int(window) == WIN, f"kernel hardcodes window={WIN}, got {window}"
    in_maps = _make_in_maps(query_seq, keys_seq, values_seq, Wq, Wo)
    res = _run(in_maps)
    return _gather(res.results)


def kernel_traced(query_seq, keys_seq, values_seq, Wq, Wo, window=WIN, **_unused):
    """Like kernel() but also returns BassKernelResults (profile/exec time)."""
    assert int(window) == WIN
    in_maps = _make_in_maps(query_seq, keys_seq, values_seq, Wq, Wo)
    res = _run(in_maps, trace=True)
    return _gather(res.results), res


# revision 50
# speedup vs baseline: 1.0011x; 1.0011x over previous
# Local (sliding-window, strictly-causal) multi-head attention for Trainium2.
#
# Problem: nn_LocalAttention  (B=2, S=4096, MD=AD=1024, NH=8, HD=128, window=256)
#   q = query @ Wq.T ; per-head scores q.k/sqrt(HD) masked to col in [row-256, row-1];
#   softmax; out = w @ v ; rows with no valid keys zeroed; out @ Wo.T.
#
# Sharding (8 cores): batch (2) x sequence chunks (4 x 1024 rows).  Each core runs
# the whole pipeline for its 1024 query rows using a 256-row K/V halo, so the 8
# output shards are disjoint and the gather is pure concatenation.  Weights are
# replicated.  All tensors are pre-laid-out host-side in their exact SBUF
# [partition, ...] shape (bf16), so every DMA is a single contiguous transfer.
#
# All matmuls run in bf16 (fp32 matmuls execute as two HW passes on TRN2 and
# forgo fast-weight-load; bf16 halves PE time and DMA bytes).  PSUM accumulation
# stays fp32.  Masking is multiplicative on the exp output (DVE) instead of
# additive -1e5 bias matmuls, which removes 144 PE matmuls.
#
# Structure: ONE pass per head — Wq projection for head h (both C-halves), then
# that head's 8 attention query tiles — followed by a single Wo phase.  This
# keeps the PE stream dense from ~10us on (no phase-1/phase-2 boundary) and
# makes every DMA deadline lenient.  DMAs are explicitly ordered with dep edges
# so early-needed transfers are not bandwidth-starved by later ones.
#
# Engine split per (head, qtile): PE 6 matmuls + 1 transpose; ACT exp +
# normalize-copy (Copy activation, per-partition scale = 1/denominator); DVE
# mask multiply + reciprocal + outT copy; GPSIMD idle (its tensor ops measured
# ~3x slower than DVE and stalled the PV critical path).

import math

import ml_dtypes
import numpy as np

import concourse.bass as bass
import concourse.tile as tile
from concourse import bacc, mybir
from concourse.bass_utils import run_bass_kernel_spmd
from concourse.masks import make_identity
from concourse.tile_rust import add_dep_helper

F32 = mybir.dt.float32
BF16 = mybir.dt.bfloat16
NPBF16 = ml_dtypes.bfloat16

NH = 8       # heads
HD = 128     # head dim
B = 2        # batch
S = 4096     # sequence
MD = 1024    # model dim
AD = 1024    # attn dim
WIN = 256    # window
C = 1024     # query rows per core (chunk)
NQT = C // 128          # 8 query tiles per chunk
HALO = WIN + C          # 1280 key/value rows per core
NKB = HALO // 128       # 10 key blocks
NCORES = 8


# ----------------------------------------------------------------------------
# device program
# ----------------------------------------------------------------------------

def _emit(ctx, tc: tile.TileContext, qc, wq, wo, kT, vp, msk, out):
    nc = tc.nc

    const_pool = ctx.enter_context(tc.tile_pool(name="const", bufs=1))
    ident = const_pool.tile([128, 128], F32)
    make_identity(nc, ident)

    kT_pool = ctx.enter_context(tc.tile_pool(name="kT", bufs=1))
    msk_pool = ctx.enter_context(tc.tile_pool(name="msk", bufs=1))
    qT_pool = ctx.enter_context(tc.tile_pool(name="qT", bufs=1))
    vp_pool = ctx.enter_context(tc.tile_pool(name="vp", bufs=1))
    wo_pool = ctx.enter_context(tc.tile_pool(name="wo", bufs=1))
    outT_pool = ctx.enter_context(tc.tile_pool(name="outT", bufs=1))
    qc_pool = ctx.enter_context(tc.tile_pool(name="qc", bufs=1))
    wq_pool = ctx.enter_context(tc.tile_pool(name="wq", bufs=1))

    qc_sb = qc_pool.tile([128, 2, 8, 512], BF16)
    wq_sb = wq_pool.tile([128, NH, 8, 128], BF16)
    kT_sb = kT_pool.tile([128, NH, HALO], BF16)
    msk_sb = msk_pool.tile([128, 3, 3, 128], BF16)
    vp_sb = vp_pool.tile([128, NH, NKB, HD + 1], BF16)
    wo_sb = wo_pool.tile([128, NH, MD], BF16)
    qT_sb = qT_pool.tile([128, NH, C], BF16)
    outT_sb = outT_pool.tile([128, NH, NQT, 128], BF16)

    # DMAs, dependency-chained so arrival order tracks the compute deadlines:
    #   {qc half0, wq h0} -> qc half1 -> msk -> kv_h0 -> wq h1-7 -> kv_h1 -> ...
    d_qc0 = nc.sync.dma_start(out=qc_sb[:, 0], in_=qc[:, 0])
    d_wq0 = nc.sync.dma_start(out=wq_sb[:, 0:1], in_=wq[:, 0:1])
    d_qc1 = nc.sync.dma_start(out=qc_sb[:, 1], in_=qc[:, 1])
    d_msk = nc.sync.dma_start(out=msk_sb, in_=msk)
    d_kv = []
    for h in range(NH):
        d_k = nc.sync.dma_start(out=kT_sb[:, h], in_=kT[:, h])
        d_v = nc.sync.dma_start(out=vp_sb[:, h], in_=vp[:, h])
        d_kv.append((d_k, d_v))
    d_wqr = nc.sync.dma_start(out=wq_sb[:, 1:NH], in_=wq[:, 1:NH])
    d_wo = nc.sync.dma_start(out=wo_sb, in_=wo)

    chain = [[d_qc1, d_wqr], list(d_kv[0]) + [d_msk], list(d_kv[1]) + list(d_kv[2]),
             list(d_kv[3]) + list(d_kv[4]), list(d_kv[5]) + list(d_kv[6]),
             list(d_kv[7]) + [d_wo]]
    prev = [d_qc0, d_wq0]
    for grp in chain:
        for dn in grp:
            for dp in prev:
                add_dep_helper(dn.ins, dp.ins, True, "dma deadline order")
        prev = grp

    # ---------------- per-head: Wq projection then attention -----------------
    with tc.tile_pool(name="qp_psum", bufs=2, space="PSUM") as qp_psum, \
         tc.tile_pool(name="sc_psum", bufs=2, space="PSUM") as sc_psum, \
         tc.tile_pool(name="ov_psum", bufs=2, space="PSUM") as ov_psum, \
         tc.tile_pool(name="tr_psum", bufs=2, space="PSUM") as tr_psum, \
         tc.tile_pool(name="e", bufs=4) as e_pool, \
         tc.tile_pool(name="em", bufs=2) as em_pool, \
         tc.tile_pool(name="oh", bufs=3) as oh_pool, \
         tc.tile_pool(name="r", bufs=3) as r_pool:

        for h in range(NH):
            for half in range(2):
                ps = qp_psum.tile([128, 512], F32)
                for mt in range(8):
                    nc.tensor.matmul(
                        ps,
                        lhsT=wq_sb[:, h, mt, :],
                        rhs=qc_sb[:, half, mt, :],
                        start=(mt == 0),
                        stop=(mt == 7),
                    )
                if half == 0:
                    nc.scalar.copy(
                        qT_sb[:, h, half * 512:(half + 1) * 512], ps)
                else:
                    nc.vector.tensor_copy(
                        qT_sb[:, h, half * 512:(half + 1) * 512], ps)

            # scores block-major: one matmul per key block j covers every
            # query tile that looks at it (qt in [j-2, j]), amortizing the
            # kT weight load 3x and cutting the matmul count 24 -> 10 per head.
            em_all = em_pool.tile([128, NKB, 3, 128], BF16)
            for j in range(NKB):
                lo = max(0, j - 2)
                hi = min(NQT, j + 1)
                width = (hi - lo) * 128
                s_ps = sc_psum.tile([128, 3 * 128], F32, tag="s")
                nc.tensor.matmul(
                    s_ps[:, 0:width],
                    lhsT=kT_sb[:, h, j * 128:(j + 1) * 128],
                    rhs=qT_sb[:, h, lo * 128:hi * 128],
                    start=True,
                    stop=True,
                )
                e_sb = e_pool.tile([128, 3 * 128], BF16, tag="e")
                nc.scalar.activation(
                    e_sb[:, 0:width],
                    s_ps[:, 0:width],
                    mybir.ActivationFunctionType.Exp,
                )
                # blocks >=2 share one window pattern; only blocks 0/1
                # are core-dependent (k_abs >= 0 at the sequence start)
                nc.vector.tensor_mul(
                    em_all[:, j].rearrange("p a b -> p (a b)")[:, 0:width],
                    e_sb[:, 0:width],
                    msk_sb[:, min(j, 2)].rearrange("p a b -> p (a b)")[:, 0:width],
                )

            def flush(pqt, poh):
                # transpose of tile pqt, emitted one iteration late so the PE
                # never waits on the recip/normalize chain of the same tile
                t_ps = tr_psum.tile([128, 128], F32, name="t_ps", tag="t")
                nc.tensor.transpose(t_ps, poh, ident)
                nc.vector.tensor_copy(outT_sb[:, h, pqt, :], t_ps)

            pending = None
            for qt in range(NQT):
                # out[t, d] plus the softmax denominator in column 128
                o_ps = ov_psum.tile([128, HD + 1], F32)
                for sub in range(3):
                    j = qt + sub
                    idx = qt - max(0, j - 2)
                    nc.tensor.matmul(
                        o_ps,
                        lhsT=em_all[:, j, idx, :],
                        rhs=vp_sb[:, h, j, :],
                        start=(sub == 0),
                        stop=(sub == 2),
                    )
                r_sb = r_pool.tile([128, 1], F32)
                nc.vector.reciprocal(r_sb, o_ps[:, HD:HD + 1])
                oh_sb = oh_pool.tile([128, 128], F32)
                if qt % 2 == 0:
                    nc.scalar.activation(
                        oh_sb,
                        o_ps[:, 0:HD],
                        mybir.ActivationFunctionType.Copy,
                        scale=r_sb[:, 0:1],
                    )
                else:
                    nc.vector.tensor_scalar_mul(oh_sb, o_ps[:, 0:HD], r_sb)
                if pending is not None:
                    flush(*pending)
                pending = (qt, oh_sb)
            flush(*pending)

    # ---------------- output projection --------------------------------------
    with tc.tile_pool(name="stage", bufs=2) as stage_pool, \
         tc.tile_pool(name="fi_psum", bufs=2, space="PSUM") as fi_psum:
        for qt in range(NQT):
            f_ps = fi_psum.tile([128, MD], F32)
            for h in range(NH):
                lhsT = outT_sb[:, h, qt, :]
                for nn in range(2):
                    nc.tensor.matmul(
                        f_ps[:, nn * 512:(nn + 1) * 512],
                        lhsT=lhsT,
                        rhs=wo_sb[:, h, nn * 512:(nn + 1) * 512],
                        start=(h == 0),
                        stop=(h == NH - 1),
                    )
            st = stage_pool.tile([128, MD], BF16)
            nc.vector.tensor_copy(st, f_ps)
            nc.sync.dma_start(out=out[qt * 128:(qt + 1) * 128, :], in_=st)


_CACHED_NC = {}


def _build_program():
    if "nc" in _CACHED_NC:
        return _CACHED_NC["nc"]
    nc = bacc.Bacc("TRN2", target_bir_lowering=False, debug=False)
    qc = nc.dram_tensor("qc", [128, 2, 8, 512], BF16, kind="ExternalInput").ap()
    wq = nc.dram_tensor("wq", [128, NH, 8, 128], BF16, kind="ExternalInput").ap()
    wo = nc.dram_tensor("wo", [128, NH, MD], BF16, kind="ExternalInput").ap()
    kT = nc.dram_tensor("kT", [128, NH, HALO], BF16, kind="ExternalInput").ap()
    vp = nc.dram_tensor("vp", [128, NH, NKB, HD + 1], BF16, kind="ExternalInput").ap()
    msk = nc.dram_tensor("msk", [128, 3, 3, 128], BF16, kind="ExternalInput").ap()
    out = nc.dram_tensor("out", [C, MD], BF16, kind="ExternalOutput").ap()
    from contextlib import ExitStack

    with tile.TileContext(nc) as tc:
        with ExitStack() as ctx:
            _emit(ctx, tc, qc, wq, wo, kT, vp, msk, out)
    nc.compile()
    _CACHED_NC["nc"] = nc
    return nc


# ----------------------------------------------------------------------------
# host-side shard construction
# ----------------------------------------------------------------------------

def _build_mask(s0: int) -> np.ndarray:
    """Multiplicative 0/1 window mask, block-major: [k, kblock j, slot, t].

    Block j's score matmul covers query tiles qt in [max(0, j-2), min(8, j+1));
    slot idx maps to qt = max(0, j-2) + idx."""
    m = np.zeros((128, 3, 3, 128), np.float32)
    tt = np.arange(128)
    kk = np.arange(128)
    for j, jj in [(0, 0), (1, 1), (4, 2)]:   # block 4 = shared interior pattern
        lo = max(0, j - 2)
        hi = min(NQT, j + 1)
        k_abs = s0 + j * 128 - WIN + kk                 # [k]
        for idx in range(hi - lo):
            qt = lo + idx
            q_abs = s0 + qt * 128 + tt                  # [t]
            valid = (
                (k_abs[:, None] < q_abs[None, :])
                & (q_abs[None, :] - k_abs[:, None] <= WIN)
                & (k_abs[:, None] >= 0)
            )
            m[:, jj, idx, :][valid] = 1.0
    if s0 == 0:
        # row 0 has no valid keys; give it one unmasked zero-padding key so
        # softmax yields weight 1 on v=0 -> output row is exactly 0, matching
        # the reference's has_valid zeroing.  (qt=0, sub=0 -> block 0, slot 0.)
        m[0, 0, 0, 0] = 1.0
    return m.astype(NPBF16)


def _make_in_maps(query_seq, keys_seq, values_seq, Wq, Wo):
    q = np.asarray(query_seq, dtype=np.float32)
    k = np.asarray(keys_seq, dtype=np.float32)
    v = np.asarray(values_seq, dtype=np.float32)
    wq_f = np.asarray(Wq, dtype=np.float32)
    wo_f = np.asarray(Wo, dtype=np.float32)

    scale = np.float32(math.sqrt(float(HD)))
    # wq[p, h, mt, j] = Wq[h*128+j, mt*128+p] / scale  (h-major for early start)
    wq_h = np.ascontiguousarray(
        (wq_f.T / scale).reshape(8, 128, NH, 128).transpose(1, 2, 0, 3)
    ).astype(NPBF16)
    # wo[d, h, o] = Wo[o, h*128+d]
    wo_h = np.ascontiguousarray(
        wo_f.T.reshape(NH, 128, MD).transpose(1, 0, 2)).astype(NPBF16)

    masks = {0: _build_mask(0)}

    in_maps = []
    for core in range(NCORES):
        b, ch = divmod(core, S // C)
        s0 = ch * C

        # qc[p, half, mt, t'] = q[b, s0+half*512+t', mt*128+p]
        qc_h = np.ascontiguousarray(
            q[b, s0:s0 + C, :].reshape(2, 512, 8, 128).transpose(3, 0, 2, 1)
        ).astype(NPBF16)

        khalo = np.zeros((HALO, AD), np.float32)
        vhalo = np.zeros((HALO, AD), np.float32)
        lo = s0 - WIN
        off = max(0, -lo)
        khalo[off:] = k[b, lo + off:s0 + C, :]
        vhalo[off:] = v[b, lo + off:s0 + C, :]

        # kT[d, h, j] = khalo[j, h*128+d]
        kT_h = np.ascontiguousarray(
            khalo.reshape(HALO, NH, HD).transpose(2, 1, 0)).astype(NPBF16)

        # vp[p, h, blk, d] = vhalo[blk*128+p, h*128+d]; ones at d=HD
        vp_h = np.ones((128, NH, NKB, HD + 1), np.float32)
        vp_h[:, :, :, :HD] = vhalo.reshape(NKB, 128, NH, HD).transpose(1, 2, 0, 3)
        vp_h = np.ascontiguousarray(vp_h).astype(NPBF16)

        if s0 not in masks:
            masks[s0] = _build_mask(s0)

        in_maps.append({
            "qc": qc_h,
            "wq": wq_h,
            "wo": wo_h,
            "kT": kT_h,
            "vp": vp_h,
            "msk": masks[s0],
        })
    return in_maps


def _gather(results) -> np.ndarray:
    out = np.empty((B, S, MD), np.float32)
    for core in range(NCORES):
        b, ch = divmod(core, S // C)
        out[b, ch * C:(ch + 1) * C, :] = results[core]["out"].astype(np.float32)
    return out


def _run(in_maps, **kwargs):
    nc = _build_program()
    return run_bass_kernel_spmd(nc, in_maps, list(range(NCORES)), **kwargs)


def kernel(query_seq, keys_seq, values_seq, Wq, Wo, window=WIN, **_unused):
    assert int(window) == WIN, f"kernel hardcodes window={WIN}, got {window}"
    in_maps = _make_in_maps(query_seq, keys_seq, values_seq, Wq, Wo)
    res = _run(in_maps)
    return _gather(res.results)


def kernel_traced(query_seq, keys_seq, values_seq, Wq, Wo, window=WIN, **_unused):
    """Like kernel() but also returns BassKernelResults (profile/exec time)."""
    assert int(window) == WIN
    in_maps = _make_in_maps(query_seq, keys_seq, values_seq, Wq, Wo)
    res = _run(in_maps, trace=True)
    return _gather(res.results), res


# revision 52
# speedup vs baseline: 1.0322x; 1.0310x over previous
# Local (sliding-window, strictly-causal) multi-head attention for Trainium2.
#
# Problem: nn_LocalAttention  (B=2, S=4096, MD=AD=1024, NH=8, HD=128, window=256)
#   q = query @ Wq.T ; per-head scores q.k/sqrt(HD) masked to col in [row-256, row-1];
#   softmax; out = w @ v ; rows with no valid keys zeroed; out @ Wo.T.
#
# Sharding (8 cores): batch (2) x sequence chunks (4 x 1024 rows).  Each core runs
# the whole pipeline for its 1024 query rows using a 256-row K/V halo, so the 8
# output shards are disjoint and the gather is pure concatenation.  Weights are
# replicated.  All tensors are pre-laid-out host-side in their exact SBUF
# [partition, ...] shape (bf16), so every DMA is a single contiguous transfer.
#
# All matmuls run in bf16 (fp32 matmuls execute as two HW passes on TRN2 and
# forgo fast-weight-load; bf16 halves PE time and DMA bytes).  PSUM accumulation
# stays fp32.  Masking is multiplicative on the exp output (DVE) instead of
# additive -1e5 bias matmuls, which removes 144 PE matmuls.
#
# Structure: ONE pass per head — Wq projection for head h (both C-halves), then
# that head's 8 attention query tiles — followed by a single Wo phase.  This
# keeps the PE stream dense from ~10us on (no phase-1/phase-2 boundary) and
# makes every DMA deadline lenient.  DMAs are explicitly ordered with dep edges
# so early-needed transfers are not bandwidth-starved by later ones.
#
# Engine split per (head, qtile): PE 6 matmuls + 1 transpose; ACT exp +
# normalize-copy (Copy activation, per-partition scale = 1/denominator); DVE
# mask multiply + reciprocal + outT copy; GPSIMD idle (its tensor ops measured
# ~3x slower than DVE and stalled the PV critical path).

import math

import ml_dtypes
import numpy as np

import concourse.bass as bass
import concourse.tile as tile
from concourse import bacc, mybir
from concourse.bass_utils import run_bass_kernel_spmd
from concourse.masks import make_identity
from concourse.tile_rust import add_dep_helper

F32 = mybir.dt.float32
BF16 = mybir.dt.bfloat16
NPBF16 = ml_dtypes.bfloat16

NH = 8       # heads
HD = 128     # head dim
B = 2        # batch
S = 4096     # sequence
MD = 1024    # model dim
AD = 1024    # attn dim
WIN = 256    # window
C = 1024     # query rows per core (chunk)
NQT = C // 128          # 8 query tiles per chunk
HALO = WIN + C          # 1280 key/value rows per core
NKB = HALO // 128       # 10 key blocks
NCORES = 8


# ----------------------------------------------------------------------------
# device program
# ----------------------------------------------------------------------------

def _emit(ctx, tc: tile.TileContext, qc, wq, wo, kT, vp, msk, out):
    nc = tc.nc

    const_pool = ctx.enter_context(tc.tile_pool(name="const", bufs=1))
    ident = const_pool.tile([128, 128], F32)
    make_identity(nc, ident)

    kT_pool = ctx.enter_context(tc.tile_pool(name="kT", bufs=1))
    msk_pool = ctx.enter_context(tc.tile_pool(name="msk", bufs=1))
    qT_pool = ctx.enter_context(tc.tile_pool(name="qT", bufs=1))
    vp_pool = ctx.enter_context(tc.tile_pool(name="vp", bufs=1))
    wo_pool = ctx.enter_context(tc.tile_pool(name="wo", bufs=1))
    outT_pool = ctx.enter_context(tc.tile_pool(name="outT", bufs=1))
    qc_pool = ctx.enter_context(tc.tile_pool(name="qc", bufs=1))
    wq_pool = ctx.enter_context(tc.tile_pool(name="wq", bufs=1))

    qc_sb = qc_pool.tile([128, 2, 8, 512], BF16)
    wq_sb = wq_pool.tile([128, NH, 8, 128], BF16)
    kT_sb = kT_pool.tile([128, NH, HALO], BF16)
    msk_sb = msk_pool.tile([128, 3, 3, 128], BF16)
    vp_sb = vp_pool.tile([128, NH, NKB, HD + 1], BF16)
    wo_sb = wo_pool.tile([128, NH, MD], BF16)
    qT_sb = qT_pool.tile([128, NH, C], BF16)
    outT_sb = outT_pool.tile([128, NH, NQT, 128], BF16)

    # DMAs, dependency-chained so arrival order tracks the compute deadlines:
    #   {qc half0, wq h0} -> qc half1 -> msk -> kv_h0 -> wq h1-7 -> kv_h1 -> ...
    d_qc0 = nc.sync.dma_start(out=qc_sb[:, 0], in_=qc[:, 0])
    d_wq0 = nc.sync.dma_start(out=wq_sb[:, 0:1], in_=wq[:, 0:1])
    d_qc1 = nc.sync.dma_start(out=qc_sb[:, 1], in_=qc[:, 1])
    d_msk = nc.sync.dma_start(out=msk_sb, in_=msk)
    d_wq = [d_wq0] + [nc.sync.dma_start(out=wq_sb[:, h:h + 1], in_=wq[:, h:h + 1])
                      for h in range(1, NH)]
    d_kv = []
    for h in range(NH):
        d_k = nc.sync.dma_start(out=kT_sb[:, h], in_=kT[:, h])
        d_v = nc.sync.dma_start(out=vp_sb[:, h], in_=vp[:, h])
        d_kv.append((d_k, d_v))
    d_wo = nc.sync.dma_start(out=wo_sb, in_=wo)

    # alternating {kv_h, wq_h+2} groups: matches the per-head consumption rate
    # (wq every ~3.4us, kv every ~5.6us) so neither stream starves the other
    chain = [[d_qc1, d_wq[1], d_msk]]
    for h in range(NH):
        grp = list(d_kv[h])
        if h + 2 < NH:
            grp.append(d_wq[h + 2])
        chain.append(grp)
    chain.append([d_wo])
    prev = [d_qc0, d_wq0]
    for grp in chain:
        for dn in grp:
            for dp in prev:
                add_dep_helper(dn.ins, dp.ins, True, "dma deadline order")
        prev = grp

    # ---------------- per-head: Wq projection then attention -----------------
    with tc.tile_pool(name="qp_psum", bufs=2, space="PSUM") as qp_psum, \
         tc.tile_pool(name="sc_psum", bufs=2, space="PSUM") as sc_psum, \
         tc.tile_pool(name="ov_psum", bufs=2, space="PSUM") as ov_psum, \
         tc.tile_pool(name="tr_psum", bufs=2, space="PSUM") as tr_psum, \
         tc.tile_pool(name="e", bufs=4) as e_pool, \
         tc.tile_pool(name="em", bufs=2) as em_pool, \
         tc.tile_pool(name="oh", bufs=3) as oh_pool, \
         tc.tile_pool(name="r", bufs=3) as r_pool:

        for h in range(NH):
            for half in range(2):
                ps = qp_psum.tile([128, 512], F32)
                for mt in range(8):
                    nc.tensor.matmul(
                        ps,
                        lhsT=wq_sb[:, h, mt, :],
                        rhs=qc_sb[:, half, mt, :],
                        start=(mt == 0),
                        stop=(mt == 7),
                    )
                if half == 0:
                    nc.scalar.copy(
                        qT_sb[:, h, half * 512:(half + 1) * 512], ps)
                else:
                    nc.vector.tensor_copy(
                        qT_sb[:, h, half * 512:(half + 1) * 512], ps)

            # scores block-major: one matmul per key block j covers every
            # query tile that looks at it (qt in [j-2, j]), amortizing the
            # kT weight load 3x and cutting the matmul count 24 -> 10 per head.
            em_all = em_pool.tile([128, NKB, 3, 128], BF16)
            for j in range(NKB):
                lo = max(0, j - 2)
                hi = min(NQT, j + 1)
                width = (hi - lo) * 128
                s_ps = sc_psum.tile([128, 3 * 128], F32, tag="s")
                nc.tensor.matmul(
                    s_ps[:, 0:width],
                    lhsT=kT_sb[:, h, j * 128:(j + 1) * 128],
                    rhs=qT_sb[:, h, lo * 128:hi * 128],
                    start=True,
                    stop=True,
                )
                e_sb = e_pool.tile([128, 3 * 128], BF16, tag="e")
                nc.scalar.activation(
                    e_sb[:, 0:width],
                    s_ps[:, 0:width],
                    mybir.ActivationFunctionType.Exp,
                )
                nc.vector.tensor_mul(
                    em_all[:, j].rearrange("p a b -> p (a b)")[:, 0:width],
                    e_sb[:, 0:width],
                    msk_sb[:, min(j, 2)].rearrange("p a b -> p (a b)")[:, 0:width],
                )

            def flush(pqt, poh):
                # transpose of tile pqt, emitted one iteration late so the PE
                # never waits on the recip/normalize chain of the same tile
                t_ps = tr_psum.tile([128, 128], F32, name="t_ps", tag="t")
                nc.tensor.transpose(t_ps, poh, ident)
                nc.vector.tensor_copy(outT_sb[:, h, pqt, :], t_ps)

            pending = None
            for qt in range(NQT):
                # out[t, d] plus the softmax denominator in column 128
                o_ps = ov_psum.tile([128, HD + 1], F32)
                for sub in range(3):
                    j = qt + sub
                    idx = qt - max(0, j - 2)
                    nc.tensor.matmul(
                        o_ps,
                        lhsT=em_all[:, j, idx, :],
                        rhs=vp_sb[:, h, j, :],
                        start=(sub == 0),
                        stop=(sub == 2),
                    )
                r_sb = r_pool.tile([128, 1], F32)
                nc.vector.reciprocal(r_sb, o_ps[:, HD:HD + 1])
                oh_sb = oh_pool.tile([128, 128], F32)
                if qt % 2 == 0:
                    nc.scalar.activation(
                        oh_sb,
                        o_ps[:, 0:HD],
                        mybir.ActivationFunctionType.Copy,
                        scale=r_sb[:, 0:1],
                    )
                else:
                    nc.vector.tensor_scalar_mul(oh_sb, o_ps[:, 0:HD], r_sb)
                if pending is not None:
                    flush(*pending)
                pending = (qt, oh_sb)
            flush(*pending)

    # ---------------- output projection --------------------------------------
    with tc.tile_pool(name="stage", bufs=2) as stage_pool, \
         tc.tile_pool(name="fi_psum", bufs=2, space="PSUM") as fi_psum:
        for qt in range(NQT):
            f_ps = fi_psum.tile([128, MD], F32)
            for h in range(NH):
                lhsT = outT_sb[:, h, qt, :]
                for nn in range(2):
                    nc.tensor.matmul(
                        f_ps[:, nn * 512:(nn + 1) * 512],
                        lhsT=lhsT,
                        rhs=wo_sb[:, h, nn * 512:(nn + 1) * 512],
                        start=(h == 0),
                        stop=(h == NH - 1),
                    )
            st = stage_pool.tile([128, MD], BF16)
            nc.vector.tensor_copy(st, f_ps)
            nc.sync.dma_start(out=out[qt * 128:(qt + 1) * 128, :], in_=st)


_CACHED_NC = {}


def _build_program():
    if "nc" in _CACHED_NC:
        return _CACHED_NC["nc"]
    nc = bacc.Bacc("TRN2", target_bir_lowering=False, debug=False)
    qc = nc.dram_tensor("qc", [128, 2, 8, 512], BF16, kind="ExternalInput").ap()
    wq = nc.dram_tensor("wq", [128, NH, 8, 128], BF16, kind="ExternalInput").ap()
    wo = nc.dram_tensor("wo", [128, NH, MD], BF16, kind="ExternalInput").ap()
    kT = nc.dram_tensor("kT", [128, NH, HALO], BF16, kind="ExternalInput").ap()
    vp = nc.dram_tensor("vp", [128, NH, NKB, HD + 1], BF16, kind="ExternalInput").ap()
    msk = nc.dram_tensor("msk", [128, 3, 3, 128], BF16, kind="ExternalInput").ap()
    out = nc.dram_tensor("out", [C, MD], BF16, kind="ExternalOutput").ap()
    from contextlib import ExitStack

    with tile.TileContext(nc) as tc:
        with ExitStack() as ctx:
            _emit(ctx, tc, qc, wq, wo, kT, vp, msk, out)
    nc.compile()
    _CACHED_NC["nc"] = nc
    return nc


# ----------------------------------------------------------------------------
# host-side shard construction
# ----------------------------------------------------------------------------

def _build_mask(s0: int) -> np.ndarray:
    """Multiplicative 0/1 window mask, block-major: [k, kblock j, slot, t].

    Block j's score matmul covers query tiles qt in [max(0, j-2), min(8, j+1));
    slot idx maps to qt = max(0, j-2) + idx."""
    m = np.zeros((128, 3, 3, 128), np.float32)
    tt = np.arange(128)
    kk = np.arange(128)
    for j, jj in [(0, 0), (1, 1), (4, 2)]:   # block 4 = shared interior pattern
        lo = max(0, j - 2)
        hi = min(NQT, j + 1)
        k_abs = s0 + j * 128 - WIN + kk                 # [k]
        for idx in range(hi - lo):
            qt = lo + idx
            q_abs = s0 + qt * 128 + tt                  # [t]
            valid = (
                (k_abs[:, None] < q_abs[None, :])
                & (q_abs[None, :] - k_abs[:, None] <= WIN)
                & (k_abs[:, None] >= 0)
            )
            m[:, jj, idx, :][valid] = 1.0
    if s0 == 0:
        # row 0 has no valid keys; give it one unmasked zero-padding key so
        # softmax yields weight 1 on v=0 -> output row is exactly 0, matching
        # the reference's has_valid zeroing.  (qt=0, sub=0 -> block 0, slot 0.)
        m[0, 0, 0, 0] = 1.0
    return m.astype(NPBF16)


def _make_in_maps(query_seq, keys_seq, values_seq, Wq, Wo):
    q = np.asarray(query_seq, dtype=np.float32)
    k = np.asarray(keys_seq, dtype=np.float32)
    v = np.asarray(values_seq, dtype=np.float32)
    wq_f = np.asarray(Wq, dtype=np.float32)
    wo_f = np.asarray(Wo, dtype=np.float32)

    scale = np.float32(math.sqrt(float(HD)))
    # wq[p, h, mt, j] = Wq[h*128+j, mt*128+p] / scale  (h-major for early start)
    wq_h = np.ascontiguousarray(
        (wq_f.T / scale).reshape(8, 128, NH, 128).transpose(1, 2, 0, 3)
    ).astype(NPBF16)
    # wo[d, h, o] = Wo[o, h*128+d]
    wo_h = np.ascontiguousarray(
        wo_f.T.reshape(NH, 128, MD).transpose(1, 0, 2)).astype(NPBF16)

    masks = {0: _build_mask(0)}

    in_maps = []
    for core in range(NCORES):
        b, ch = divmod(core, S // C)
        s0 = ch * C

        # qc[p, half, mt, t'] = q[b, s0+half*512+t', mt*128+p]
        qc_h = np.ascontiguousarray(
            q[b, s0:s0 + C, :].reshape(2, 512, 8, 128).transpose(3, 0, 2, 1)
        ).astype(NPBF16)

        khalo = np.zeros((HALO, AD), np.float32)
        vhalo = np.zeros((HALO, AD), np.float32)
        lo = s0 - WIN
        off = max(0, -lo)
        khalo[off:] = k[b, lo + off:s0 + C, :]
        vhalo[off:] = v[b, lo + off:s0 + C, :]

        # kT[d, h, j] = khalo[j, h*128+d]
        kT_h = np.ascontiguousarray(
            khalo.reshape(HALO, NH, HD).transpose(2, 1, 0)).astype(NPBF16)

        # vp[p, h, blk, d] = vhalo[blk*128+p, h*128+d]; ones at d=HD
        vp_h = np.ones((128, NH, NKB, HD + 1), np.float32)
        vp_h[:, :, :, :HD] = vhalo.reshape(NKB, 128, NH, HD).transpose(1, 2, 0, 3)
        vp_h = np.ascontiguousarray(vp_h).astype(NPBF16)

        if s0 not in masks:
            masks[s0] = _build_mask(s0)

        in_maps.append({
            "qc": qc_h,
            "wq": wq_h,
            "wo": wo_h,
            "kT": kT_h,
            "vp": vp_h,
            "msk": masks[s0],
        })
    return in_maps


def _gather(results) -> np.ndarray:
    out = np.empty((B, S, MD), np.float32)
    for core in range(NCORES):
        b, ch = divmod(core, S // C)
        out[b, ch * C:(ch + 1) * C, :] = results[core]["out"].astype(np.float32)
    return out


def _run(in_maps, **kwargs):
    nc = _build_program()
    return run_bass_kernel_spmd(nc, in_maps, list(range(NCORES)), **kwargs)


def kernel(query_seq, keys_seq, values_seq, Wq, Wo, window=WIN, **_unused):
    assert int(window) == WIN, f"kernel hardcodes window={WIN}, got {window}"
    in_maps = _make_in_maps(query_seq, keys_seq, values_seq, Wq, Wo)
    res = _run(in_maps)
    return _gather(res.results)


def kernel_traced(query_seq, keys_seq, values_seq, Wq, Wo, window=WIN, **_unused):
    """Like kernel() but also returns BassKernelResults (profile/exec time)."""
    assert int(window) == WIN
    in_maps = _make_in_maps(query_seq, keys_seq, values_seq, Wq, Wo)
    res = _run(in_maps, trace=True)
    return _gather(res.results), res
